# revision 14
# baseline (speedup 1.0000x reference)
"""Trainium2 (8 NeuronCores) kernel for AdaptiveFeatureLinkedCosineLoss.

Reference math:
    link = l2norm_rows(link_matrix)          # (D, D)
    rn   = l2norm_rows(z_rna)                # (B, D)
    an   = l2norm_rows(z_atac)               # (B, D)
    cos[b] = sum_ij rn[b,i] link[i,j] an[b,j]
    ent_* = mean_b( -sum_i v ln(v + 1e-8) )  for v in {rn, an}
    tau  = clip(sig(t)*0.1 + (1-sig(t))*avg_ent, 0.01, 1.0)
    loss = -mean_b(cos[b]) / tau

Device-side refactor (per core, batch shard of 1024 rows):
    sum_b cos[b] = <Lnorm, Rn^T An> = sum_j sum_i linv[i] * L[i,j] * C[i,j]
    with C = Rn^T An contracting over the *batch* axis (natural layout, no
    transposes). The link row normalization linv rides for free as the
    stationary operand of the partition-reduce matmul that folds the i axis:
        cos_ps[1, j] += sum_i linv[i] * (C ⊙ L_raw)[i, j]
    Entropy sums use the same partition-reduce matmul with a ones vector.
    Inverse row norms are computed entirely on the vector engine with a
    bit-trick rsqrt seed + 2 Newton steps, so ScalarE runs only Square and
    Ln passes (one activation-table load each, no table thrash).
    Tensors are split into per-batch-tile / per-half tiles so the Tile
    scheduler's tile-granular dependency tracking streams the matmul as
    DMA+normalize complete, instead of waiting for whole-tensor readiness.

Each core returns [1, 2] partial sums (cos_sum, sum rn*ln + sum an*ln);
the host sums cores and applies the scalar epilogue.
"""

import numpy as np

import concourse.bass as bass
import concourse.tile as tile
from concourse import bacc, mybir
from concourse.bass_utils import run_bass_kernel_spmd

B, D = 8192, 1024
N_CORES = 8
B_LOC = B // N_CORES  # rows per core
P = 128
KT = B_LOC // P  # batch tiles per core (8)
IT = D // P  # feature tiles (8)
H = KT // 2  # half-batch group
F32 = mybir.dt.float32
I32 = mybir.dt.int32
BF16 = mybir.dt.bfloat16
EPS_LOG = 1e-8
INV_NORM_CLAMP = 1e12  # == 1 / EPS_NORM(1e-12)
TEMPERATURE_INIT = 0.1
MAGIC = 0x5F3759DF
N_LINK_ON_ACT = 4  # link sumsq tiles on ScalarE; rest via DVE bn_stats
N_WARM_MM = 16


def build_nc():
    nc = bacc.Bacc(None, target_bir_lowering=False, num_devices=N_CORES)

    z_rna = nc.dram_tensor("z_rna", [B_LOC, D], F32, kind="ExternalInput").ap()
    z_atac = nc.dram_tensor("z_atac", [B_LOC, D], F32, kind="ExternalInput").ap()
    link = nc.dram_tensor("link_matrix", [D, D], F32, kind="ExternalInput").ap()
    out = nc.dram_tensor("out", [1, 2], F32, kind="ExternalOutput").ap()

    Sq = mybir.ActivationFunctionType.Square
    LnF = mybir.ActivationFunctionType.Ln
    op = mybir.AluOpType
    mult, add = op.mult, op.add

    n_ent_mm = 2 * KT * 2  # (rna+atac) x KT tiles x 2 j-halves
    n_cos_mm = IT * 2

    with tile.TileContext(nc) as tc:
        with (
            tc.tile_pool(name="persist", bufs=1) as persist,
            tc.tile_pool(name="scratch", bufs=3) as scratch,
            tc.tile_pool(name="scratch2", bufs=2) as scratch2,
            tc.tile_pool(name="small", bufs=4) as small,
            tc.tile_pool(name="cpsum", bufs=3, space="PSUM") as cpsum,
            tc.tile_pool(name="accpsum", bufs=1, space="PSUM") as accpsum,
        ):
            # per-k raw input tiles (separate tiles -> precise deps)
            zr = [persist.tile([P, D], F32, name=f"zr{k}") for k in range(KT)]
            za = [persist.tile([P, D], F32, name=f"za{k}") for k in range(KT)]
            L = [persist.tile([P, D], F32, name=f"L{t}") for t in range(IT)]
            # normalized halves: [P, H, D] so half-width ACT/DVE ops work
            XnH = [persist.tile([P, H, D], BF16, name=f"XnH{h}") for h in range(2)]
            YnH = [persist.tile([P, H, D], BF16, name=f"YnH{h}") for h in range(2)]
            z_ssH = [persist.tile([P, H, 2], F32, name=f"zss{h}") for h in range(2)]
            z_invH = [persist.tile([P, H, 2], F32, name=f"zinv{h}") for h in range(2)]
            l_ssH = [persist.tile([P, 4], F32, name=f"lss{h}") for h in range(2)]
            l_invH = [persist.tile([P, 4], F32, name=f"linv{h}") for h in range(2)]
            l_invbfH = [persist.tile([P, 4], BF16, name=f"linvbf{h}") for h in range(2)]
            out_sb = persist.tile([1, 2], F32)
            eps_b = persist.tile([P, 1], F32)
            zero_b = persist.tile([P, 1], F32)
            ones = persist.tile([P, 1], BF16)
            warm = persist.tile([P, 512], BF16)
            nc.vector.memset(eps_b, EPS_LOG)
            nc.vector.memset(zero_b, 0.0)
            nc.vector.memset(ones, 1.0)

            ent_ps = accpsum.tile([1, 512], F32)
            cos_ps = accpsum.tile([1, 512], F32)

            def rsqrt_batch(ss_ap, inv_ap, shape):
                """inv = min(1/sqrt(ss), 1e12): quake seed + 2 Newton steps,
                all on DVE (ss=0 -> huge seed -> clamp, matching the
                reference's 1/max(norm, 1e-12) guard)."""
                y = inv_ap
                yi = y.bitcast(I32)
                t1 = small.tile(shape, F32)
                t2 = small.tile(shape, F32)
                nc.vector.tensor_scalar(
                    out=yi, in0=ss_ap.bitcast(I32), scalar1=1, scalar2=None,
                    op0=op.logical_shift_right,
                )
                nc.vector.tensor_scalar(
                    out=yi, in0=yi, scalar1=-1, scalar2=None, op0=op.bitwise_xor
                )
                nc.vector.tensor_scalar(
                    out=yi, in0=yi, scalar1=MAGIC + 1, scalar2=None, op0=op.add
                )
                for _ in range(2):
                    nc.vector.tensor_tensor(out=t1, in0=y, in1=y, op=mult)
                    nc.vector.tensor_tensor(out=t1, in0=t1, in1=ss_ap, op=mult)
                    nc.vector.tensor_scalar(
                        out=t2, in0=t1, scalar1=-0.5, scalar2=1.5, op0=mult, op1=add
                    )
                    nc.vector.tensor_tensor(out=y, in0=y, in1=t2, op=mult)
                nc.vector.tensor_scalar_min(out=y, in0=y, scalar1=INV_NORM_CLAMP)

            # ---- input DMAs: z pairs first (critical path), link after ----
            for k in range(KT):
                nc.sync.dma_start(out=zr[k], in_=z_rna[P * k : P * (k + 1), :])
                nc.sync.dma_start(out=za[k], in_=z_atac[P * k : P * (k + 1), :])
            for t in range(IT):
                nc.sync.dma_start(out=L[t], in_=link[P * t : P * (t + 1), :])

            # ---- z row sumsq on ACT (Square + accum), per-k tiles ----
            for k in range(KT):
                h, kk = divmod(k, H)
                for raw, col in ((zr[k], 0), (za[k], 1)):
                    sq = scratch.tile([P, D], F32)
                    nc.scalar.activation(
                        out=sq, in_=raw, func=Sq, bias=zero_b,
                        accum_out=z_ssH[h][:, kk, col : col + 1],
                    )

            # ---- per-half inv + normalize (streams the matmul early) ----
            for h in range(2):
                rsqrt_batch(z_ssH[h], z_invH[h], [P, H, 2])
                for kk in range(H):
                    k = H * h + kk
                    nc.vector.tensor_scalar_mul(
                        out=XnH[h][:, kk, :], in0=zr[k],
                        scalar1=z_invH[h][:, kk, 0:1],
                    )
                    nc.vector.tensor_scalar_mul(
                        out=YnH[h][:, kk, :], in0=za[k],
                        scalar1=z_invH[h][:, kk, 1:2],
                    )

            # ---- link row sumsq (needed for the cos matmuls) ----
            for t in range(N_LINK_ON_ACT):
                lsq = scratch.tile([P, D], F32)
                nc.scalar.activation(
                    out=lsq, in_=L[t], func=Sq, bias=zero_b,
                    accum_out=l_ssH[t // 4][:, t % 4 : t % 4 + 1],
                )
            for t in range(N_LINK_ON_ACT, IT):
                stats = small.tile([P, 2, nc.vector.BN_STATS_DIM], F32)
                for sub in range(2):
                    nc.vector.bn_stats(
                        out=stats[:, sub, :],
                        in_=L[t][:, 512 * sub : 512 * (sub + 1)],
                    )
                mv = small.tile([P, nc.vector.BN_AGGR_DIM], F32)
                nc.vector.bn_aggr(out=mv, in_=stats)
                # ss = (var + mean^2) * D
                msq = small.tile([P, 1], F32)
                nc.vector.tensor_tensor(
                    out=msq, in0=mv[:, 0:1], in1=mv[:, 0:1], op=mult
                )
                nc.vector.tensor_tensor(out=msq, in0=msq, in1=mv[:, 1:2], op=add)
                nc.vector.tensor_scalar_mul(
                    out=l_ssH[t // 4][:, t % 4 : t % 4 + 1], in0=msq,
                    scalar1=float(D),
                )
            for h in range(2):
                rsqrt_batch(l_ssH[h], l_invH[h], [P, 4])
                nc.vector.tensor_copy(out=l_invbfH[h], in_=l_invH[h])

            # ---- PE warmup: dummy matmuls timed to end as the C stream
            # starts, so HAM is at full clock (warm seeded from a mid-stream
            # DMA tile; garbage values, results discarded) ----
            nc.vector.tensor_copy(out=warm, in_=zr[3][:, 0:512])
            wpsum = cpsum.tile([P, D], F32, tag="cbuf")
            for i in range(N_WARM_MM):
                nc.tensor.matmul(
                    wpsum[:, 0:512], lhsT=warm[:, 0:128], rhs=warm,
                    start=True, stop=True,
                )

            # ---- C = Xn^T Yn per 128-row i-tile; consume with raw L;
            # cos partition-reduce weighted by linv as stationary operand ----
            mm_c = 0
            for t in range(IT):
                C = cpsum.tile([P, D], F32, tag="cbuf")
                for j in range(2):
                    for h in range(2):
                        for kk in range(H):
                            nc.tensor.matmul(
                                C[:, 512 * j : 512 * (j + 1)],
                                lhsT=XnH[h][:, kk, P * t : P * (t + 1)],
                                rhs=YnH[h][:, kk, 512 * j : 512 * (j + 1)],
                                start=(h == 0 and kk == 0),
                                stop=(h == 1 and kk == H - 1),
                            )
                cprod = scratch.tile([P, D], BF16)
                nc.vector.tensor_tensor(out=cprod, in0=C, in1=L[t], op=mult)
                for j in range(2):
                    nc.tensor.matmul(
                        cos_ps,
                        lhsT=l_invbfH[t // 4][:, t % 4 : t % 4 + 1],
                        rhs=cprod[:, 512 * j : 512 * (j + 1)],
                        start=(mm_c == 0),
                        stop=(mm_c == n_cos_mm - 1),
                    )
                    mm_c += 1

            # ---- entropy: half-width ln (ACT), x*ln (DVE), ones-matmul ----
            mm_i = 0
            for h in range(2):
                for nrm in (XnH[h], YnH[h]):
                    lnt = scratch2.tile([P, H, D], BF16)
                    nc.scalar.activation(out=lnt, in_=nrm, func=LnF, bias=eps_b)
                    prod = scratch2.tile([P, H, D], BF16)
                    nc.vector.tensor_tensor(out=prod, in0=nrm, in1=lnt, op=mult)
                    for kk in range(H):
                        for j in range(2):
                            nc.tensor.matmul(
                                ent_ps,
                                lhsT=ones,
                                rhs=prod[:, kk, 512 * j : 512 * (j + 1)],
                                start=(mm_i == 0),
                                stop=(mm_i == n_ent_mm - 1),
                            )
                            mm_i += 1

            # ---- finals ----
            nc.vector.tensor_reduce(
                out=out_sb[:, 0:1], in_=cos_ps, axis=mybir.AxisListType.X, op=add
            )
            nc.vector.tensor_reduce(
                out=out_sb[:, 1:2], in_=ent_ps, axis=mybir.AxisListType.X, op=add
            )
            nc.sync.dma_start(out=out, in_=out_sb)

    nc.compile()
    return nc


_NC_CACHE = None


def _get_nc():
    global _NC_CACHE
    if _NC_CACHE is None:
        _NC_CACHE = build_nc()
    return _NC_CACHE


def make_in_maps(z_rna, z_atac, link_matrix):
    z_rna = np.ascontiguousarray(np.asarray(z_rna, dtype=np.float32))
    z_atac = np.ascontiguousarray(np.asarray(z_atac, dtype=np.float32))
    link_matrix = np.ascontiguousarray(np.asarray(link_matrix, dtype=np.float32))
    return [
        {
            "z_rna": z_rna[i * B_LOC : (i + 1) * B_LOC],
            "z_atac": z_atac[i * B_LOC : (i + 1) * B_LOC],
            "link_matrix": link_matrix,
        }
        for i in range(N_CORES)
    ]


def finalize(partials, temp_param):
    """partials: [n_cores, 1, 2] device sums -> scalar loss (np.float32)."""
    p = np.asarray(partials, dtype=np.float64)
    cos_sum = p[..., 0].sum()
    avg_entropy = -(p[..., 1].sum() / (2.0 * B))
    t = np.float64(np.asarray(temp_param, dtype=np.float32))
    s = 1.0 / (1.0 + np.exp(-t))
    adaptive = s * TEMPERATURE_INIT + (1.0 - s) * avg_entropy
    tau = min(max(adaptive, 0.01), 1.0)
    loss = -(cos_sum / B) / tau
    return np.float32(loss)


def kernel(z_rna, z_atac, link_matrix, temp_param):
    nc = _get_nc()
    in_maps = make_in_maps(z_rna, z_atac, link_matrix)
    res = run_bass_kernel_spmd(nc, in_maps, core_ids=list(range(N_CORES)))
    partials = np.stack([r["out"] for r in res.results])
    return np.asarray(finalize(partials, temp_param))
